# revision 1
# baseline (speedup 1.0000x reference)
"""GENConv (GCNEncoder) 2-layer GNN on 8 Trainium2 NeuronCores.

Self-contained: kernel(**inputs) -> np.ndarray [50000, 64] float32.

Strategy (graph-partitioned, per spec sharding hint):
  - Nodes split into 8 contiguous blocks of 6250 by destination; each core
    owns its block's incident edges (grouped by dst) and computes its block
    of output nodes.
  - The gather source x (relu'd, message form) is replicated per core in a
    [50002, 64] DRAM buffer with sentinel rows (-1e30) at rows 0 and 50001;
    node i lives at row i+1.  Two int16-index windows (rows [0,32768) and
    rows [17234, 50002)) cover the full range for dma_gather.
  - Per core, nodes are permuted by (lo-degree, hi-degree) so 128-node tiles
    have near-uniform padded degrees; padded slots point at sentinel rows so
    exp() gives exactly 0 (softmax without max-subtraction; logits here are
    bounded so no overflow).
  - Softmax aggregation: e = exp(t*(g+eps)); num = sum (g+eps)*e;
    den = sum e; aggr = num/(den+1e-16).  Matches reference up to the
    placement of the 1e-16 regulariser (negligible).
  - MLP kept feature-major (h1T = W1^T @ h0^T) so BatchNorm scale/bias are
    per-partition; BN batch stats via accum_out + one tiny AllReduce.
    b1 is dropped entirely (train-mode BN is shift invariant).
  - Layer output is scatter-added (CCE) back to block order, AllGathered
    into the next layer's gather source.
"""
import os
import sys

sys.path.insert(0, "/opt/trn_rl_repo")

import numpy as np

EPS = 1e-7
BN_EPS = 1e-5
C = 64
P = 128
WIN = 32768
NEG = -1.0e30


# --------------------------------------------------------------------------
# host-side preprocessing
# --------------------------------------------------------------------------

def _wrap16(flat):
    """dma_gather/scatter index layout: element j at [j % 16, j // 16]."""
    n = len(flat)
    assert n % 16 == 0
    arr = np.zeros((128, n // 16), dtype=np.int16)
    arr[:16, :] = np.asarray(flat, dtype=np.int16).reshape(n // 16, 16).T
    return arr


class Plan:
    pass


def preprocess(x, edge_index, W1, b1, gamma, beta, W2, b2, t, ncores=8):
    tarr = np.asarray(t, dtype=np.float32)
    x = np.asarray(x, dtype=np.float32)
    N = x.shape[0]
    assert x.shape[1] == C
    src = np.asarray(edge_index[0], dtype=np.int64)
    dst = np.asarray(edge_index[1], dtype=np.int64)
    assert N % ncores == 0
    BLK = N // ncores
    NT = (BLK + P - 1) // P
    NPAD = NT * P
    ROWS = N + 1

    order = np.argsort(dst, kind="stable")
    src_s = src[order]
    dst_s = dst[order]
    bounds = np.searchsorted(dst_s, np.arange(0, N + 1, BLK))

    deg = np.zeros((ncores, BLK), dtype=np.int64)
    core_edges = []
    for d in range(ncores):
        sl = slice(bounds[d], bounds[d + 1])
        s_ = src_s[sl]
        ld = dst_s[sl] - d * BLK
        deg[d] = np.bincount(ld, minlength=BLK)
        core_edges.append((s_, ld))

    perms = [np.argsort(deg[d], kind="stable") for d in range(ncores)]
    G = np.zeros(NT, dtype=np.int64)
    for d in range(ncores):
        kp = np.zeros(NPAD, dtype=np.int64)
        kp[:BLK] = deg[d][perms[d]]
        G = np.maximum(G, kp.reshape(NT, P).max(axis=1))

    # gather groups: consecutive tiles, capped total slots
    SLOT_CAP = 64
    groups = []
    cur = []
    cur_sum = 0
    for t in range(NT):
        g = int(G[t])
        if cur and (cur_sum + g > SLOT_CAP or len(cur) >= 6):
            groups.append(cur)
            cur = []
            cur_sum = 0
        cur.append(t)
        cur_sum += g
    if cur:
        groups.append(cur)

    colbase = []
    c = 0
    for grp in groups:
        colbase.append(c)
        c += int(sum(int(G[t]) for t in grp))
    CL = c

    plan = Plan()
    plan.N, plan.ncores, plan.BLK, plan.NT, plan.NPAD = N, ncores, BLK, NT, NPAD
    plan.ROWS = ROWS
    plan.G = G
    plan.groups = groups
    plan.colbase = colbase
    plan.CL = CL

    off_in_grp = np.zeros(NT, dtype=np.int64)
    grp_of = np.zeros(NT, dtype=np.int64)
    for gi, grp in enumerate(groups):
        o = 0
        for t in grp:
            off_in_grp[t] = o
            grp_of[t] = gi
            o += int(G[t])

    in_maps = []
    for d in range(ncores):
        s_, ld = core_edges[d]
        perm = perms[d]
        perm_pos = np.empty(BLK, dtype=np.int64)
        perm_pos[perm] = np.arange(BLK)
        pos = perm_pos[ld]
        o2 = np.argsort(pos, kind="stable")
        s2 = s_[o2]
        pos2 = pos[o2]
        cnt = np.bincount(pos2, minlength=NPAD)
        start = np.concatenate([[0], np.cumsum(cnt)[:-1]])
        rank = np.arange(len(s2)) - start[pos2]

        gidx = np.zeros((128, max(CL, 1)), dtype=np.int32)
        tl2 = pos2 // P
        p2 = pos2 % P
        cols = np.asarray(colbase)[grp_of[tl2]] + off_in_grp[tl2] + rank
        gidx[p2, cols] = (s2 + 1).astype(np.int32)

        # layer-2 indices: xr1 holds permuted blocks [core, NPAD]; map each
        # source to core*NPAD + its position in that core's permutation.
        # Pad slots (gidx==0) point at this core's own pad row (filled -1e30).
        all_pos = np.empty(N, dtype=np.int64)
        for dd in range(ncores):
            pp_ = np.empty(BLK, dtype=np.int64)
            pp_[perms[dd]] = np.arange(BLK)
            all_pos[dd * BLK:(dd + 1) * BLK] = dd * NPAD + pp_
        gidx1 = np.zeros((128, max(CL, 1)), dtype=np.int32)
        gidx1[p2, cols] = all_pos[s2].astype(np.int32)
        padslot = gidx == 0
        # slot (p,col) unused -> row 0 valid for l0 (sentinel); for l1 use a
        # pad row of block d (position BLK of this core's permuted block)
        gidx1[padslot] = d * NPAD + BLK
        gidx1[p2, cols] = all_pos[s2].astype(np.int32)  # re-apply real ones

        xop = np.zeros((NPAD, C), dtype=np.float32)
        xop[:BLK] = x[d * BLK:(d + 1) * BLK][perm]

        m = {
            "x_full": x,
            "x_own_perm": xop,
            "gidx": gidx,
            "gidx1": gidx1,
            "tsc": np.repeat(tarr[:, None], 128, axis=1),
            "tbi": np.repeat((tarr * EPS)[:, None], 128, axis=1),
            "W1": np.ascontiguousarray(W1, dtype=np.float32),
            "W2": np.ascontiguousarray(W2, dtype=np.float32),
            "gam": np.ascontiguousarray(gamma, dtype=np.float32),
            "bet": np.ascontiguousarray(beta, dtype=np.float32),
            "b2r": np.repeat(np.asarray(b2, np.float32)[:, None, :], 128, axis=1),
            "tmask": (np.arange(128) < (BLK - (NT - 1) * P)).astype(np.float32)[:, None],
            "nmask": np.where(np.arange(128) < (BLK - (NT - 1) * P), 0.0,
                              NEG).astype(np.float32)[:, None].repeat(C, axis=1),
        }
        in_maps.append(m)
    plan.perms = perms
    return plan, in_maps


# --------------------------------------------------------------------------
# device program
# --------------------------------------------------------------------------

def build(plan):
    import concourse.bacc as bacc
    import concourse.bass as bass
    import concourse.tile as tile
    from concourse import mybir
    from concourse.masks import make_identity

    N, ncores, BLK, NT, NPAD = plan.N, plan.ncores, plan.BLK, plan.NT, plan.NPAD
    ROWS = plan.ROWS
    G = plan.G
    groups = plan.groups
    CL = plan.CL
    F = 2 * C
    AF = mybir.ActivationFunctionType
    AL = mybir.AluOpType
    IOA = bass.IndirectOffsetOnAxis

    nc = bacc.Bacc("TRN2", target_bir_lowering=False, debug=False,
                   num_devices=ncores)
    f32 = mybir.dt.float32
    i32 = mybir.dt.int32

    x_full = nc.dram_tensor("x_full", [N, C], f32, kind="ExternalInput")
    x_own_perm = nc.dram_tensor("x_own_perm", [NPAD, C], f32, kind="ExternalInput")
    gidx_i = nc.dram_tensor("gidx", [128, max(CL, 1)], i32, kind="ExternalInput")
    gidx1_i = nc.dram_tensor("gidx1", [128, max(CL, 1)], i32, kind="ExternalInput")
    tsc_i = nc.dram_tensor("tsc", [2, 128], f32, kind="ExternalInput")
    tbi_i = nc.dram_tensor("tbi", [2, 128], f32, kind="ExternalInput")
    W1_i = nc.dram_tensor("W1", [2, C, F], f32, kind="ExternalInput")
    W2_i = nc.dram_tensor("W2", [2, F, C], f32, kind="ExternalInput")
    gam_i = nc.dram_tensor("gam", [2, F], f32, kind="ExternalInput")
    bet_i = nc.dram_tensor("bet", [2, F], f32, kind="ExternalInput")
    b2r_i = nc.dram_tensor("b2r", [2, 128, C], f32, kind="ExternalInput")
    tmask_i = nc.dram_tensor("tmask", [128, 1], f32, kind="ExternalInput")
    nmask_i = nc.dram_tensor("nmask", [128, C], f32, kind="ExternalInput")
    y2 = nc.dram_tensor("y2", [NPAD, C], f32, kind="ExternalOutput")

    RG = [list(range(ncores))]
    TAIL = BLK - (NT - 1) * P

    with tile.TileContext(nc) as tc:
        with (
            tc.tile_pool(name="const", bufs=1) as cp,
            tc.tile_pool(name="big", bufs=1) as bp,
            tc.tile_pool(name="relu", bufs=2) as rp,
            tc.tile_pool(name="gath", bufs=2) as gp,
            tc.tile_pool(name="fin", bufs=2) as fp_,
            tc.tile_pool(name="mlp", bufs=3) as mp,
            tc.tile_pool(name="psum", bufs=2, space="PSUM") as pp,
            tc.tile_pool(name="dram", bufs=1, space="DRAM") as dp,
        ):
            shared = "Shared" if ncores > 4 else "Local"
            xr0 = dp.tile([ROWS, C], f32, tag="xr0")
            xr1 = dp.tile([ncores * NPAD, C], f32, tag="xr1", addr_space=shared)
            y1d = dp.tile([NPAD, C], f32, tag="y1d")
            st_ins = [dp.tile([F, 2], f32, tag=f"st_in{l}", name=f"st_in{l}")
                      for l in range(2)]
            st_outs = [dp.tile([F, 2], f32, tag=f"st_out{l}", name=f"st_out{l}",
                               addr_space=shared) for l in range(2)]

            # ---------------- constants / params ----------------
            gidx_s = cp.tile([128, max(CL, 1)], i32)
            nc.sync.dma_start(gidx_s[:], gidx_i[:])
            gidx1_s = cp.tile([128, max(CL, 1)], i32)
            nc.sync.dma_start(gidx1_s[:], gidx1_i[:])
            tmask = cp.tile([128, 1], f32)
            nc.sync.dma_start(tmask[:], tmask_i[:])
            nmask = cp.tile([128, C], f32)
            nc.sync.dma_start(nmask[:], nmask_i[:])

            tscs, tbis, W1s, W2s, gams, bets, b2rs = [], [], [], [], [], [], []
            for l in range(2):
                a = cp.tile([128, 1], f32, tag="tsc")
                nc.sync.dma_start(a[:], tsc_i[l, :, None])
                tscs.append(a)
                a = cp.tile([128, 1], f32, tag="tbi")
                nc.sync.dma_start(a[:], tbi_i[l, :, None])
                tbis.append(a)
                a = cp.tile([C, F], f32, tag="w1")
                nc.sync.dma_start(a[:], W1_i[l])
                W1s.append(a)
                a = cp.tile([F, C], f32, tag="w2")
                nc.sync.dma_start(a[:], W2_i[l])
                W2s.append(a)
                a = cp.tile([F, 1], f32, tag="gam")
                nc.sync.dma_start(a[:], gam_i[l, :, None])
                gams.append(a)
                a = cp.tile([F, 1], f32, tag="bet")
                nc.sync.dma_start(a[:], bet_i[l, :, None])
                bets.append(a)
                a = cp.tile([128, C], f32, tag="b2r")
                nc.sync.dma_start(a[:], b2r_i[l])
                b2rs.append(a)

            ident = cp.tile([P, P], f32)
            make_identity(nc, ident[:])
            dumrow = cp.tile([1, C], f32)
            nc.vector.memset(dumrow[:], NEG)

            # persistent big buffers; resid-like tiles share one slot
            h_store = bp.tile([128, NT * F], f32)
            resid0 = bp.tile([128, NT, C], f32, tag="resid")
            y1s = bp.tile([128, NT, C], f32, tag="resid")
            y2s = bp.tile([128, NT, C], f32, tag="resid")
            hsum_cols = bp.tile([F, NT], f32)
            sq_cols = bp.tile([F, NT], f32)

            # ---------------- prologue ----------------
            nc.sync.dma_start(xr0[0:1, :], dumrow[:])
            # relu pass: xr0 rows 1..N  <- relu(x_full)
            xf = x_full.ap().rearrange("a c -> (a c)")
            x0f = xr0[:].rearrange("a c -> (a c)")
            tot = N * C
            CH_E = 128 * 1600
            assert tot % 128 == 0
            offz = 0
            while offz < tot:
                n = min(CH_E, tot - offz)
                buf = rp.tile([128, CH_E // 128], f32, tag="relu")
                nc.sync.dma_start(buf[:, :n // 128],
                                  xf[offz:offz + n].rearrange("(p f) -> p f", p=128))
                nc.scalar.activation(buf[:, :n // 128], buf[:, :n // 128], AF.Relu)
                nc.sync.dma_start(x0f[C + offz:C + offz + n].rearrange(
                    "(p f) -> p f", p=128), buf[:, :n // 128])
                offz += n
            # residual load (layer 1), already permuted on host
            nc.sync.dma_start(
                resid0[:], x_own_perm.ap().rearrange("(t p) c -> p t c", p=128))

            # ---------------- layers ----------------
            for l in range(2):
                xr = xr0 if l == 0 else xr1
                gsrc = gidx_s if l == 0 else gidx1_s
                resid = resid0 if l == 0 else y1s
                ybuf = y1s if l == 0 else y2s
                ydst = y1d if l == 0 else y2

                for gi, grp in enumerate(groups):
                    SG = sum(int(G[t]) for t in grp)
                    GT = len(grp)
                    gb = gp.tile([128, max(SG, 1), C], f32, tag="gb")
                    eb = gp.tile([128, max(SG, 1), C], f32, tag="eb")
                    for sl in range(SG):
                        col = plan.colbase[gi] + sl
                        nc.gpsimd.indirect_dma_start(
                            out=gb[:, sl, :], out_offset=None, in_=xr[:],
                            in_offset=IOA(ap=gsrc[:, col:col + 1], axis=0))
                    if SG:
                        nc.scalar.activation(eb[:, :SG, :], gb[:, :SG, :], AF.Exp,
                                             bias=tbis[l][:], scale=tscs[l][:])
                        nc.vector.scalar_tensor_tensor(
                            gb[:, :SG, :], gb[:, :SG, :], float(EPS), eb[:, :SG, :],
                            op0=AL.add, op1=AL.mult)

                    dn = fp_.tile([128, GT, C], f32, tag="dn")
                    nm = fp_.tile([128, GT, C], f32, tag="nm")
                    for tr, t in enumerate(grp):
                        g_t = int(G[t])
                        o_t = sum(int(G[q]) for q in grp[:tr])
                        if g_t:
                            nc.vector.tensor_reduce(
                                dn[:, tr, :], eb[:, o_t:o_t + g_t, :].rearrange("p s c -> p c s"),
                                axis=mybir.AxisListType.X, op=AL.add)
                            nc.vector.tensor_reduce(
                                nm[:, tr, :], gb[:, o_t:o_t + g_t, :].rearrange("p s c -> p c s"),
                                axis=mybir.AxisListType.X, op=AL.add)
                        else:
                            nc.vector.memset(dn[:, tr, :], 0.0)
                            nc.vector.memset(nm[:, tr, :], 0.0)

                    # den += 1e-16 ; rec = 1/den ; aggr = num*rec ; h0 = aggr+resid
                    nc.vector.tensor_scalar_add(dn[:, :GT, :], dn[:, :GT, :], 1e-16)
                    nc.vector.reciprocal(dn[:, :GT, :], dn[:, :GT, :])
                    nc.vector.tensor_tensor(nm[:, :GT, :], nm[:, :GT, :],
                                            dn[:, :GT, :], op=AL.mult)
                    h0g = fp_.tile([128, GT, C], f32, tag="h0g")
                    t0 = grp[0]
                    nc.vector.tensor_tensor(h0g[:, :GT, :], nm[:, :GT, :],
                                            resid[:, t0:t0 + GT, :], op=AL.add)

                    for tr, t in enumerate(grp):
                        if t == NT - 1 and TAIL < P:
                            nc.vector.tensor_scalar_mul(h0g[:, tr, :],
                                                        h0g[:, tr, :], tmask[:])
                        h0tp = pp.tile([C, P], f32, tag="h0tp", space="PSUM")
                        nc.tensor.transpose(h0tp[:], h0g[:, tr, :], ident[:])
                        h0t = mp.tile([C, P], f32, tag="h0t")
                        nc.scalar.copy(h0t[:], h0tp[:])
                        h1p = pp.tile([F, P], f32, tag="h1p", space="PSUM")
                        nc.tensor.matmul(h1p[:], W1s[l][:], h0t[:],
                                         start=True, stop=True)
                        nc.scalar.activation(
                            h_store[:, t * F:(t + 1) * F], h1p[:], AF.Identity,
                            accum_out=hsum_cols[:, t:t + 1])
                        sqs = mp.tile([F, P], f32, tag="sqs")
                        nc.scalar.activation(sqs[:], h1p[:], AF.Square,
                                             accum_out=sq_cols[:, t:t + 1])

                # ---- BN stats all-reduce ----
                stt = mp.tile([F, 2], f32, tag="stt")
                nc.vector.tensor_reduce(stt[:, 0:1], hsum_cols[:],
                                        axis=mybir.AxisListType.X, op=AL.add)
                nc.vector.tensor_reduce(stt[:, 1:2], sq_cols[:],
                                        axis=mybir.AxisListType.X, op=AL.add)
                nc.sync.dma_start(st_ins[l][:], stt[:])
                nc.gpsimd.collective_compute(
                    "AllReduce", AL.add, replica_groups=RG,
                    ins=[st_ins[l][:]], outs=[st_outs[l][:]])
                stb = mp.tile([F, 2], f32, tag="stb")
                nc.sync.dma_start(stb[:], st_outs[l][:])
                mean = mp.tile([F, 1], f32, tag="mean")
                nc.scalar.mul(mean[:], stb[:, 0:1], 1.0 / N)
                msq = mp.tile([F, 1], f32, tag="msq")
                nc.scalar.mul(msq[:], stb[:, 1:2], 1.0 / N)
                sqm = mp.tile([F, 1], f32, tag="sqm")
                nc.scalar.activation(sqm[:], mean[:], AF.Square)
                var = mp.tile([F, 1], f32, tag="var")
                nc.vector.scalar_tensor_tensor(var[:], msq[:], float(BN_EPS), sqm[:],
                                               op0=AL.add, op1=AL.subtract)
                std = mp.tile([F, 1], f32, tag="std")
                nc.scalar.activation(std[:], var[:], AF.Sqrt)
                rstd = mp.tile([F, 1], f32, tag="rstd")
                nc.vector.reciprocal(rstd[:], std[:])
                scl = mp.tile([F, 1], f32, tag="scl")
                nc.vector.tensor_tensor(scl[:], rstd[:], gams[l][:], op=AL.mult)
                tmp = mp.tile([F, 1], f32, tag="tmp")
                nc.vector.tensor_tensor(tmp[:], mean[:], scl[:], op=AL.mult)
                shf = mp.tile([F, 1], f32, tag="shf")
                nc.vector.tensor_tensor(shf[:], bets[l][:], tmp[:], op=AL.subtract)

                # ---- phase 2: BN+relu, mm2, +b2, relu ----
                for t in range(NT):
                    h2t = mp.tile([F, P], f32, tag="h2t")
                    nc.scalar.activation(
                        h2t[:], h_store[:, t * F:(t + 1) * F], AF.Relu,
                        bias=shf[:], scale=scl[:])
                    yp = pp.tile([P, C], f32, tag="yp", space="PSUM")
                    nc.tensor.matmul(yp[:], h2t[:], W2s[l][:], start=True, stop=True)
                    nc.vector.tensor_tensor(ybuf[:, t, :], yp[:], b2rs[l][:],
                                            op=AL.add)
                    nc.scalar.activation(ybuf[:, t, :], ybuf[:, t, :], AF.Relu)

                # ---- writeback in permuted order (plain DMA) ----
                if l == 0:
                    # pad rows -> -1e30 so layer-2 pad gathers hit a sentinel:
                    # y = y*tmask + nmask  (tmask 1/0, nmask 0/-1e30)
                    nc.vector.scalar_tensor_tensor(
                        ybuf[:, NT - 1, :], ybuf[:, NT - 1, :], tmask[:],
                        nmask[:], op0=AL.mult, op1=AL.add)
                nc.sync.dma_start(
                    ydst[:].rearrange("(t p) c -> p t c", p=128), ybuf[:])
                if l == 0:
                    nc.gpsimd.collective_compute(
                        "AllGather", AL.bypass, replica_groups=RG,
                        ins=[y1d[:]], outs=[xr1[:]])

    nc.compile()
    return nc


# --------------------------------------------------------------------------
# entry point
# --------------------------------------------------------------------------

def run(inputs, trace=False, ncores=8, **trace_kwargs):
    from concourse import bass_utils

    plan, in_maps = preprocess(**inputs, ncores=ncores)
    nc = build(plan)
    res = bass_utils.run_bass_kernel_spmd(nc, in_maps, list(range(ncores)),
                                          trace=trace, **trace_kwargs)
    out = np.empty((plan.N, C), dtype=np.float32)
    for d in range(ncores):
        blk = np.empty((plan.BLK, C), dtype=np.float32)
        blk[plan.perms[d]] = res.results[d]["y2"][:plan.BLK]
        out[d * plan.BLK:(d + 1) * plan.BLK] = blk
    return out, res


def kernel(x, edge_index, W1, b1, gamma, beta, W2, b2, t):
    out, _ = run(dict(x=x, edge_index=edge_index, W1=W1, b1=b1, gamma=gamma,
                      beta=beta, W2=W2, b2=b2, t=t))
    return out

